# revision 3
# baseline (speedup 1.0000x reference)
"""LocallyConnected2d (B=8, C_in=32, 48x48, C_out=32, 3x3, pad 1) on 8 trn2 cores.

Strategy: shard the spatial-location axis L = H*W across cores (6 image rows
each). Per location l the op is an (8x288)@(288x32) GEMM with location-unique
weights; weight streaming (85 MB total) dominates -> memory-bound.

Device mapping per core:
  - x halo slice lives in SBUF replicated 3x with kw column shifts, laid out
    [p=(kw*32+c), (row, col, b)], so the im2col patch for any location is a
    plain strided AP slice (no patch materialization).
  - Contraction (d=288) is split into 3 kh-rounds of K=96=(3 kw x 32 c),
    PSUM-accumulated. Round 0 carries a 97th row: ones in x, transposed bias
    in W, folding the bias add into the matmul.
  - Per location: stationary = x-view [K,8(b)] (8-column LDW, cheap),
    moving = W slice [K,32(o)], out = PSUM [8(b),32(o)].
  - W streamed in 2-image-row tiles (~1.2 MB DMAs) alternating between the
    two HWDGE rings (sync/scalar) for overlap; output kept in (r,q,o) layout
    so PSUM->SBUF copies are contiguous, final NCHW transpose done on host.
"""

import numpy as np

import concourse.bacc as bacc
import concourse.tile as tile
from concourse import mybir
from concourse.bass_utils import run_bass_kernel_spmd

B, C_IN, H, W = 8, 32, 48, 48
C_OUT = 32
N_CORES = 8
RP = H // N_CORES  # rows per core (6)
LP = RP * W  # locations per core (288)
RG = 2  # image rows per W tile
F32 = mybir.dt.float32

_nc = None


def _build():
    nc = bacc.Bacc(
        "TRN2", target_bir_lowering=False, debug=False, num_devices=N_CORES
    )
    xh = nc.dram_tensor("xh", [C_IN, RP + 2, W + 2, B], F32, kind="ExternalInput")
    w = nc.dram_tensor("w", [C_IN * 9, LP, C_OUT], F32, kind="ExternalInput")
    bt = nc.dram_tensor("bt", [LP, C_OUT], F32, kind="ExternalInput")
    # (b, r, q, o) layout so device-side stores are contiguous; host transposes.
    out = nc.dram_tensor("out", [B, RP, W, C_OUT], F32, kind="ExternalOutput")

    # w rows are d = c*9 + kh*3 + kw; expose (kh, kw, c) so one DMA per
    # (kh, row-group) lands as SBUF partitions p = kw*32 + c.
    wr = w.rearrange("(c kh kw) l o -> kh kw c l o", c=C_IN, kh=3, kw=3)

    dma_engines = None

    with tile.TileContext(nc) as tc:
        with (
            tc.tile_pool(name="xpool", bufs=1) as xpool,
            tc.tile_pool(name="wpool", bufs=6) as wpool,
            tc.tile_pool(name="opool", bufs=1) as opool,
            tc.tile_pool(name="pspool", bufs=8, space="PSUM") as pspool,
        ):
            dma_engines = [nc.gpsimd, nc.gpsimd]

            x3 = xpool.tile([128, (RP + 2) * W * B], F32)
            for kw in range(3):
                dma_engines[kw % 2].dma_start(
                    x3[kw * 32 : (kw + 1) * 32, :], xh[:, :, kw : kw + W, :]
                )
            nc.vector.memset(x3[96:97, :], 1.0)

            out_sb = opool.tile([B, RP * W * C_OUT], F32)

            ndma = 0
            for g in range(RP // RG):
                wts = []
                for kh in range(3):
                    wt = wpool.tile([128, RG * W * C_OUT], F32, tag="wt")
                    dma_engines[ndma % 2].dma_start(
                        wt[0:96, :],
                        wr[kh, :, :, g * RG * W : (g + 1) * RG * W, :],
                    )
                    ndma += 1
                    if kh == 0:
                        dma_engines[ndma % 2].dma_start(
                            wt[96:97, :], bt[g * RG * W : (g + 1) * RG * W, :]
                        )
                        ndma += 1
                    wts.append(wt)
                for rl in range(g * RG, (g + 1) * RG):
                    for qg in range(W // 16):
                        ps = pspool.tile([B, 512], F32)
                        for qq in range(16):
                            q = qg * 16 + qq
                            ll = (rl - g * RG) * W + q  # loc within W tile
                            for kh in range(3):
                                kd = 97 if kh == 0 else 96
                                off = ((rl + kh) * W + q) * B
                                nc.tensor.matmul(
                                    ps[0:B, qq * 32 : (qq + 1) * 32],
                                    x3[0:kd, off : off + B],
                                    wts[kh][0:kd, ll * C_OUT : (ll + 1) * C_OUT],
                                    start=(kh == 0),
                                    stop=(kh == 2),
                                )
                        base = (rl * W + qg * 16) * C_OUT
                        nc.vector.tensor_copy(
                            out_sb[0:B, base : base + 512], ps[0:B, :]
                        )
            nc.gpsimd.dma_start(out[:, :, :, :], out_sb[0:B, :])
    nc.compile()
    return nc


def _shard(inputs):
    x = np.asarray(inputs["x"], np.float32)
    weight = np.asarray(inputs["weight"], np.float32)[0]
    bias = np.asarray(inputs["bias"], np.float32)[0]
    xp = np.pad(x, ((0, 0), (0, 0), (1, 1), (1, 1)))
    bias_t = np.ascontiguousarray(bias.reshape(C_OUT, H * W).T)
    in_maps = []
    for k in range(N_CORES):
        r0 = RP * k
        in_maps.append(
            {
                "xh": np.ascontiguousarray(
                    xp[:, :, r0 : r0 + RP + 2, :].transpose(1, 2, 3, 0)
                ),
                "w": np.ascontiguousarray(weight[:, LP * k : LP * (k + 1), :]),
                "bt": np.ascontiguousarray(bias_t[LP * k : LP * (k + 1), :]),
            }
        )
    return in_maps


def _get_nc():
    global _nc
    if _nc is None:
        _nc = _build()
    return _nc


def _gather(results):
    # per-core out is (B, RP, W, C_OUT); concat rows, then to NCHW
    full = np.concatenate([results[k]["out"] for k in range(N_CORES)], axis=1)
    return np.ascontiguousarray(full.transpose(0, 3, 1, 2))


def kernel(**inputs):
    nc = _get_nc()
    res = run_bass_kernel_spmd(nc, _shard(inputs), list(range(N_CORES)))
    return _gather(res.results)


# revision 4
# speedup vs baseline: 1.2744x; 1.2744x over previous
"""LocallyConnected2d (B=8, C_in=32, 48x48, C_out=32, 3x3, pad 1) on 8 trn2 cores.

Strategy: shard the spatial-location axis L = H*W across cores (6 image rows
each). Per location l the op is an (8x288)@(288x32) GEMM with location-unique
weights; weight streaming (85 MB total) dominates -> memory-bound.

Device mapping per core:
  - x halo slice lives in SBUF replicated 3x with kw column shifts, laid out
    [p=(kw*32+c), (row, col, b)], so the im2col patch for any location is a
    plain strided AP slice (no patch materialization). The replication, the
    kw shifts, and a 97th all-ones partition are baked in on the host so the
    device sees a single contiguous DMA.
  - Contraction (d=288) is split into 3 kh-rounds of K=96=(3 kw x 32 c),
    PSUM-accumulated. Round 0 carries a 97th row: ones in x, transposed bias
    in W, folding the bias add into the matmul.
  - Per location: stationary = x-view [K,8(b)] (8-column LDW, cheap),
    moving = W slice [K,32(o)], out = PSUM [8(b),32(o)].
  - W is host-permuted into per-(kh, 2-image-row-group) tiles that are fully
    contiguous in HBM (~1.2 MB each) so SWDGE DMAs hit line rate; output is
    kept in (r,q,o) layout so PSUM->SBUF copies are contiguous, with the
    final NCHW transpose done on host.
"""

import numpy as np

import concourse.bacc as bacc
import concourse.tile as tile
from concourse import mybir
from concourse.bass_utils import run_bass_kernel_spmd

B, C_IN, H, W = 8, 32, 48, 48
C_OUT = 32
N_CORES = 8
RP = H // N_CORES  # rows per core (6)
LP = RP * W  # locations per core (288)
RG = 2  # image rows per W tile
NG = RP // RG  # row groups per core (3)
LG = RG * W  # locations per W tile (96)
XF = (RP + 2) * W * B  # x3 free size (3072)
WF = LG * C_OUT  # W tile free size (3072)
F32 = mybir.dt.float32

_nc = None


def _build():
    nc = bacc.Bacc(
        "TRN2", target_bir_lowering=False, debug=False, num_devices=N_CORES
    )
    # x3h: kw-replicated, shifted x halo + ones row; contiguous [97, XF]
    x3h = nc.dram_tensor("x3h", [97, XF], F32, kind="ExternalInput")
    # w0: kh=0 tiles with bias row appended: [NG, 97, WF]
    w0 = nc.dram_tensor("w0", [NG, 97, WF], F32, kind="ExternalInput")
    # w12: kh=1,2 tiles: [2, NG, 96, WF]
    w12 = nc.dram_tensor("w12", [2, NG, 96, WF], F32, kind="ExternalInput")
    # (b, r, q, o) layout so device-side stores are contiguous; host transposes.
    out = nc.dram_tensor("out", [B, RP, W, C_OUT], F32, kind="ExternalOutput")

    with tile.TileContext(nc) as tc:
        with (
            tc.tile_pool(name="xpool", bufs=1) as xpool,
            tc.tile_pool(name="wpool", bufs=6) as wpool,
            tc.tile_pool(name="opool", bufs=1) as opool,
            tc.tile_pool(name="pspool", bufs=8, space="PSUM") as pspool,
        ):
            x3 = xpool.tile([128, XF], F32)
            nc.gpsimd.dma_start(x3[0:97, :], x3h[:, :])

            out_sb = opool.tile([B, RP * W * C_OUT], F32)

            for g in range(NG):
                wts = []
                for kh in range(3):
                    wt = wpool.tile([128, WF], F32, tag="wt")
                    if kh == 0:
                        nc.gpsimd.dma_start(wt[0:97, :], w0[g, :, :])
                    else:
                        nc.gpsimd.dma_start(wt[0:96, :], w12[kh - 1, g, :, :])
                    wts.append(wt)
                for rl in range(g * RG, (g + 1) * RG):
                    for qg in range(W // 16):
                        ps = pspool.tile([B, 512], F32)
                        for qq in range(16):
                            q = qg * 16 + qq
                            ll = (rl - g * RG) * W + q  # loc within W tile
                            for kh in range(3):
                                kd = 97 if kh == 0 else 96
                                off = ((rl + kh) * W + q) * B
                                nc.tensor.matmul(
                                    ps[0:B, qq * 32 : (qq + 1) * 32],
                                    x3[0:kd, off : off + B],
                                    wts[kh][0:kd, ll * C_OUT : (ll + 1) * C_OUT],
                                    start=(kh == 0),
                                    stop=(kh == 2),
                                )
                        base = (rl * W + qg * 16) * C_OUT
                        nc.vector.tensor_copy(
                            out_sb[0:B, base : base + 512], ps[0:B, :]
                        )
            nc.gpsimd.dma_start(out[:, :, :, :], out_sb[0:B, :])
    nc.compile()
    return nc


def _shard(inputs):
    x = np.asarray(inputs["x"], np.float32)
    weight = np.asarray(inputs["weight"], np.float32)[0]
    bias = np.asarray(inputs["bias"], np.float32)[0]
    xp = np.pad(x, ((0, 0), (0, 0), (1, 1), (1, 1)))
    bias_t = bias.reshape(C_OUT, H * W).T  # (L, C_OUT)

    # weight rows d = c*9 + kh*3 + kw -> [kh, g, (kw c), (l_in_g o)]
    wr = weight.reshape(C_IN, 3, 3, H * W, C_OUT)  # (c, kh, kw, l, o)

    in_maps = []
    for k in range(N_CORES):
        r0 = RP * k
        l0 = LP * k

        # x: kw-shifted triplicate + ones row
        xh = xp[:, :, r0 : r0 + RP + 2, :].transpose(1, 2, 3, 0)  # (c,r,q+2,b)
        x3h = np.empty((97, RP + 2, W, B), np.float32)
        for kw in range(3):
            x3h[kw * 32 : (kw + 1) * 32] = xh[:, :, kw : kw + W, :]
        x3h[96] = 1.0

        # weights: per (kh, g) contiguous tiles with partitions p=(kw*32+c)
        wk = wr[:, :, :, l0 : l0 + LP, :].reshape(
            C_IN, 3, 3, NG, LG, C_OUT
        )  # (c, kh, kw, g, lg, o)
        wperm = wk.transpose(1, 3, 2, 0, 4, 5)  # (kh, g, kw, c, lg, o)
        w0 = np.empty((NG, 97, WF), np.float32)
        w0[:, 0:96, :] = wperm[0].reshape(NG, 96, WF)
        w0[:, 96, :] = bias_t[l0 : l0 + LP, :].reshape(NG, WF)
        w12 = np.ascontiguousarray(wperm[1:3].reshape(2, NG, 96, WF))

        in_maps.append(
            {
                "x3h": x3h.reshape(97, XF),
                "w0": w0,
                "w12": w12,
            }
        )
    return in_maps


def _get_nc():
    global _nc
    if _nc is None:
        _nc = _build()
    return _nc


def _gather(results):
    # per-core out is (B, RP, W, C_OUT); concat rows, then to NCHW
    full = np.concatenate([results[k]["out"] for k in range(N_CORES)], axis=1)
    return np.ascontiguousarray(full.transpose(0, 3, 1, 2))


def kernel(**inputs):
    nc = _get_nc()
    res = run_bass_kernel_spmd(nc, _shard(inputs), list(range(N_CORES)))
    return _gather(res.results)


# revision 15
# speedup vs baseline: 2.1534x; 1.6898x over previous
"""LocallyConnected2d (B=8, C_in=32, 48x48, C_out=32, 3x3, pad 1) on 8 trn2 cores.

Strategy: shard the spatial-location axis L = H*W across cores (6 image rows
each). Per location l the op is an (8x288)@(288x32) GEMM with location-unique
weights; weight streaming (85 MB total) dominates -> memory-bound.

Device mapping per core:
  - x halo slice lives in SBUF replicated 3x with kw column shifts, laid out
    [p=(kw*32+c), (row, col, b)], so the im2col patch for any location is a
    plain strided AP slice (no patch materialization).
  - Contraction (d=288) is split into 3 kh-rounds of K=96=(3 kw x 32 c),
    PSUM-accumulated. K=96 everywhere keeps one PE tiling mode (no drains);
    mixed-K designs either mode-switch per matmul or hit the "row tiles
    sharing a PSUM bank" hardware fault.
  - 4 consecutive locations are column-packed onto the PE with
    tile_position=(0, 32j): stationary = x-view [96, 8(b)] into column group
    j, moving = W slice [96, 32(o)], out = PSUM partitions 32j..32j+8. The
    four matmuls per (m, kh) execute concurrently on disjoint column groups.
  - Bias is added by one K=96 matmul per (group, j): a host-baked one-hot
    column picks the group's row out of a [96, 512] bias table (rows >=18
    zeroed on device), so the op shares the (128, 32) tiling mode.
  - W is host-permuted into per-(kh, LG-location) tiles that are fully
    contiguous in HBM with 9216-byte partition rows ([96, 9216B] DMAs
    measured ~193 GB/s vs ~94 GB/s naive); output is a [128, *] fp32 tile
    ((j,b) partitions x (group, m, o) free) stored with one fast DMA and
    unscrambled to NCHW on the host.
"""

import numpy as np

import concourse.bacc as bacc
import concourse.tile as tile
from concourse import mybir
from concourse.bass_utils import run_bass_kernel_spmd

B, C_IN, H, W = 8, 32, 48, 48
C_OUT = 32
N_CORES = 8
RP = H // N_CORES  # rows per core (6)
LP = RP * W  # locations per core (288)
NGRP = LP // 16  # 16-loc output groups per core (18)

DT16 = False  # fp16 operand path (halves weight traffic)
DT = mybir.dt.float16 if DT16 else mybir.dt.float32
NPDT = np.float16 if DT16 else np.float32
LG = 144 if DT16 else 96  # locs per W tile (9216B / 12KB partition rows)
NT = LP // LG  # W tiles per kh round
XF = (RP + 2) * W * B  # x3 free size (3072)
F32 = mybir.dt.float32

_nc = None


def _build():
    nc = bacc.Bacc(
        "TRN2", target_bir_lowering=False, debug=False, num_devices=N_CORES
    )
    x3h = nc.dram_tensor("x3h", [96, XF], DT, kind="ExternalInput")
    wts_d = nc.dram_tensor("w", [NT, 3, 96, LG * C_OUT], DT, kind="ExternalInput")
    bi = nc.dram_tensor("bi", [NGRP, 512], DT, kind="ExternalInput")
    oneh = nc.dram_tensor("oneh", [96, NGRP * 8], DT, kind="ExternalInput")
    out = nc.dram_tensor("out", [128, NGRP * 128], F32, kind="ExternalOutput")

    with tile.TileContext(nc) as tc:
        with (
            tc.tile_pool(name="xpool", bufs=1) as xpool,
            tc.tile_pool(name="wpool", bufs=6) as wpool,
            tc.tile_pool(name="opool", bufs=1) as opool,
            tc.tile_pool(name="pspool", bufs=8, space="PSUM") as pspool,
        ):
            x3 = xpool.tile([96, XF], DT, tag="x3")
            nc.gpsimd.dma_start(x3[:, :], x3h[:, :])
            oneh_sb = xpool.tile([96, NGRP * 8], DT, tag="oneh")
            nc.gpsimd.dma_start(oneh_sb[:, :], oneh[:, :])
            bi_sb = xpool.tile([96, 512], DT, tag="bi")
            nc.vector.memset(bi_sb[0:96, :], 0.0)
            nc.gpsimd.dma_start(bi_sb[0:NGRP, :], bi[:, :])

            out_sb = opool.tile([128, NGRP * 128], F32)
            nc.gpsimd.memset(out_sb[:, :], 0.0)

            for t in range(NT):
                wts = []
                for kh in range(3):
                    wt = wpool.tile([96, LG * C_OUT], DT, tag="wt")
                    nc.gpsimd.dma_start(wt[:, :], wts_d[t, kh, :, :])
                    wts.append(wt)
                for gl in range(LG // 16):
                    gi = t * (LG // 16) + gl
                    rl, qg = divmod(gi, 3)
                    ps = pspool.tile([128, 128], F32)
                    for m in range(4):
                        for kh in range(3):
                            for j in range(4):
                                q = qg * 16 + m * 4 + j
                                l = rl * W + q
                                ll = l - t * LG
                                off = ((rl + kh) * W + q) * B
                                nc.tensor.matmul(
                                    ps[32 * j : 32 * j + B, m * 32 : (m + 1) * 32],
                                    x3[0:96, off : off + B],
                                    wts[kh][0:96, ll * 32 : (ll + 1) * 32],
                                    start=(m == 0 and kh == 0),
                                    stop=False,
                                    skip_group_check=True,
                                    tile_position=(0, 32 * j),
                                )
                    for j in range(4):
                        nc.tensor.matmul(
                            ps[32 * j : 32 * j + B, 0:128],
                            oneh_sb[0:96, gi * 8 : gi * 8 + 8],
                            bi_sb[0:96, j * 128 : (j + 1) * 128],
                            start=False,
                            stop=True,
                            skip_group_check=True,
                            tile_position=(0, 32 * j),
                        )
                    for j in range(4):
                        if j % 2 == 0:
                            nc.vector.tensor_copy(
                                out_sb[32 * j : 32 * j + B, gi * 128 : (gi + 1) * 128],
                                ps[32 * j : 32 * j + B, :],
                            )
                        else:
                            nc.scalar.copy(
                                out_sb[32 * j : 32 * j + B, gi * 128 : (gi + 1) * 128],
                                ps[32 * j : 32 * j + B, :],
                            )
            nc.gpsimd.dma_start(out[:, :], out_sb[0:128, :])
    nc.compile()
    return nc


def _shard(inputs):
    x = np.asarray(inputs["x"], np.float32)
    weight = np.asarray(inputs["weight"], np.float32)[0]
    bias = np.asarray(inputs["bias"], np.float32)[0]
    xp = np.pad(x, ((0, 0), (0, 0), (1, 1), (1, 1)))  # (b, c, 50, 50)
    bias_t = bias.reshape(C_OUT, H * W).T  # (L, C_OUT)
    wflat = weight.reshape(C_IN, 3, 3, H * W, C_OUT)  # (c, kh, kw, l, o)

    # one-hot group selector [96, NGRP*8]
    oneh = np.zeros((96, NGRP * 8), NPDT)
    for gi in range(NGRP):
        oneh[gi, gi * 8 : (gi + 1) * 8] = 1.0

    in_maps = []
    for k in range(N_CORES):
        r0 = RP * k
        l0 = LP * k

        x3h = np.empty((3, C_IN, RP + 2, W, B), np.float32)
        for kw in range(3):
            x3h[kw] = xp[:, :, r0 : r0 + RP + 2, kw : kw + W].transpose(1, 2, 3, 0)

        # W: [t, kh, (kw c), (lg o)]
        wk = wflat[:, :, :, l0 : l0 + LP, :].reshape(
            C_IN, 3, 3, NT, LG, C_OUT
        )  # (c, kh, kw, t, lg, o)
        wperm = wk.transpose(3, 1, 2, 0, 4, 5)  # (t, kh, kw, c, lg, o)
        wtile = wperm.reshape(NT, 3, 96, LG * C_OUT)

        # bias rows per group: (j, m, o)
        bk = bias_t[l0 : l0 + LP, :].reshape(NGRP, 4, 4, C_OUT)  # (gi, m, j, o)
        bi = bk.transpose(0, 2, 1, 3).reshape(NGRP, 512)  # (gi, (j, m, o))

        in_maps.append(
            {
                "x3h": x3h.reshape(96, XF).astype(NPDT),
                "w": np.ascontiguousarray(wtile).astype(NPDT),
                "bi": np.ascontiguousarray(bi).astype(NPDT),
                "oneh": oneh,
            }
        )
    return in_maps


def _get_nc():
    global _nc
    if _nc is None:
        _nc = _build()
    return _nc


def _gather(results):
    # out rows 32j+b (b<8) hold y[b, o, r, q] at col gi*128 + m*32 + o,
    # with r = gi//3, q = (gi%3)*16 + m*4 + j
    y = np.empty((B, C_OUT, H, W), np.float32)
    for k in range(N_CORES):
        arr = results[k]["out"].reshape(4, 32, NGRP, 4, C_OUT)  # (j, b*, gi, m, o)
        arr = arr[:, 0:B]  # (j, b, gi, m, o)
        arr = arr.transpose(1, 4, 2, 3, 0)  # (b, o, gi, m, j)
        arr = arr.reshape(B, C_OUT, RP, 3, 4, 4)  # (b, o, r, qg, m, j)
        y[:, :, RP * k : RP * (k + 1), :] = arr.reshape(B, C_OUT, RP, W)
    return y


def kernel(**inputs):
    nc = _get_nc()
    res = run_bass_kernel_spmd(nc, _shard(inputs), list(range(N_CORES)))
    return _gather(res.results)


# revision 16
# speedup vs baseline: 3.5553x; 1.6510x over previous
"""LocallyConnected2d (B=8, C_in=32, 48x48, C_out=32, 3x3, pad 1) on 8 trn2 cores.

Strategy: shard the spatial-location axis L = H*W across cores (6 image rows
each). Per location l the op is an (8x288)@(288x32) GEMM with location-unique
weights; weight streaming (85 MB total) dominates -> memory-bound.

Device mapping per core:
  - x halo slice lives in SBUF replicated 3x with kw column shifts, laid out
    [p=(kw*32+c), (row, col, b)], so the im2col patch for any location is a
    plain strided AP slice (no patch materialization).
  - Contraction (d=288) is split into 3 kh-rounds of K=96=(3 kw x 32 c),
    PSUM-accumulated. K=96 everywhere keeps one PE tiling mode (no drains);
    mixed-K designs either mode-switch per matmul or hit the "row tiles
    sharing a PSUM bank" hardware fault.
  - 4 consecutive locations are column-packed onto the PE with
    tile_position=(0, 32j): stationary = x-view [96, 8(b)] into column group
    j, moving = W slice [96, 32(o)], out = PSUM partitions 32j..32j+8. The
    four matmuls per (m, kh) execute concurrently on disjoint column groups.
  - Bias is added by one K=96 matmul per (group, j): a host-baked one-hot
    column picks the group's row out of a [96, 512] bias table (rows >=18
    zeroed on device), so the op shares the (128, 32) tiling mode.
  - W is host-permuted into per-(kh, LG-location) tiles that are fully
    contiguous in HBM with 9216-byte partition rows ([96, 9216B] DMAs
    measured ~193 GB/s vs ~94 GB/s naive); output is a [128, *] fp32 tile
    ((j,b) partitions x (group, m, o) free) stored with one fast DMA and
    unscrambled to NCHW on the host.
"""

import numpy as np

import concourse.bacc as bacc
import concourse.tile as tile
from concourse import mybir
from concourse.bass_utils import run_bass_kernel_spmd

B, C_IN, H, W = 8, 32, 48, 48
C_OUT = 32
N_CORES = 8
RP = H // N_CORES  # rows per core (6)
LP = RP * W  # locations per core (288)
NGRP = LP // 16  # 16-loc output groups per core (18)

DT16 = True  # fp16 operand path (halves weight traffic)
DT = mybir.dt.float16 if DT16 else mybir.dt.float32
NPDT = np.float16 if DT16 else np.float32
LG = 144 if DT16 else 96  # locs per W tile (9216B / 12KB partition rows)
NT = LP // LG  # W tiles per kh round
XF = (RP + 2) * W * B  # x3 free size (3072)
F32 = mybir.dt.float32

_nc = None


def _build():
    nc = bacc.Bacc(
        "TRN2", target_bir_lowering=False, debug=False, num_devices=N_CORES
    )
    x3h = nc.dram_tensor("x3h", [96, XF], DT, kind="ExternalInput")
    wts_d = nc.dram_tensor("w", [NT, 3, 96, LG * C_OUT], DT, kind="ExternalInput")
    bi = nc.dram_tensor("bi", [NGRP, 512], DT, kind="ExternalInput")
    oneh = nc.dram_tensor("oneh", [96, NGRP * 8], DT, kind="ExternalInput")
    out = nc.dram_tensor("out", [128, NGRP * 128], F32, kind="ExternalOutput")

    with tile.TileContext(nc) as tc:
        with (
            tc.tile_pool(name="xpool", bufs=1) as xpool,
            tc.tile_pool(name="wpool", bufs=6) as wpool,
            tc.tile_pool(name="opool", bufs=1) as opool,
            tc.tile_pool(name="pspool", bufs=8, space="PSUM") as pspool,
        ):
            x3 = xpool.tile([96, XF], DT, tag="x3")
            nc.gpsimd.dma_start(x3[:, :], x3h[:, :])
            oneh_sb = xpool.tile([96, NGRP * 8], DT, tag="oneh")
            nc.gpsimd.dma_start(oneh_sb[:, :], oneh[:, :])
            bi_sb = xpool.tile([96, 512], DT, tag="bi")
            nc.vector.memset(bi_sb[0:96, :], 0.0)
            nc.gpsimd.dma_start(bi_sb[0:NGRP, :], bi[:, :])

            out_sb = opool.tile([128, NGRP * 128], F32)
            nc.gpsimd.memset(out_sb[:, :], 0.0)

            for t in range(NT):
                wts = []
                for kh in range(3):
                    wt = wpool.tile([96, LG * C_OUT], DT, tag="wt")
                    nc.gpsimd.dma_start(wt[:, :], wts_d[t, kh, :, :])
                    wts.append(wt)
                for gl in range(LG // 16):
                    gi = t * (LG // 16) + gl
                    rl, qg = divmod(gi, 3)
                    ps = pspool.tile([128, 128], F32)
                    for m in range(4):
                        for kh in range(3):
                            for j in range(4):
                                q = qg * 16 + m * 4 + j
                                l = rl * W + q
                                ll = l - t * LG
                                off = ((rl + kh) * W + q) * B
                                nc.tensor.matmul(
                                    ps[32 * j : 32 * j + B, m * 32 : (m + 1) * 32],
                                    x3[0:96, off : off + B],
                                    wts[kh][0:96, ll * 32 : (ll + 1) * 32],
                                    start=(m == 0 and kh == 0),
                                    stop=False,
                                    skip_group_check=True,
                                    tile_position=(0, 32 * j),
                                )
                    for j in range(4):
                        nc.tensor.matmul(
                            ps[32 * j : 32 * j + B, 0:128],
                            oneh_sb[0:96, gi * 8 : gi * 8 + 8],
                            bi_sb[0:96, j * 128 : (j + 1) * 128],
                            start=False,
                            stop=True,
                            skip_group_check=True,
                            tile_position=(0, 32 * j),
                        )
                    for j in range(4):
                        if j % 2 == 0:
                            nc.vector.tensor_copy(
                                out_sb[32 * j : 32 * j + B, gi * 128 : (gi + 1) * 128],
                                ps[32 * j : 32 * j + B, :],
                            )
                        else:
                            nc.scalar.copy(
                                out_sb[32 * j : 32 * j + B, gi * 128 : (gi + 1) * 128],
                                ps[32 * j : 32 * j + B, :],
                            )
            nc.gpsimd.dma_start(out[:, :], out_sb[0:128, :])
    nc.compile()
    return nc


def _shard(inputs):
    x = np.asarray(inputs["x"], np.float32)
    weight = np.asarray(inputs["weight"], np.float32)[0]
    bias = np.asarray(inputs["bias"], np.float32)[0]
    xp = np.pad(x, ((0, 0), (0, 0), (1, 1), (1, 1)))  # (b, c, 50, 50)
    bias_t = bias.reshape(C_OUT, H * W).T  # (L, C_OUT)
    wflat = weight.reshape(C_IN, 3, 3, H * W, C_OUT)  # (c, kh, kw, l, o)

    # one-hot group selector [96, NGRP*8]
    oneh = np.zeros((96, NGRP * 8), NPDT)
    for gi in range(NGRP):
        oneh[gi, gi * 8 : (gi + 1) * 8] = 1.0

    in_maps = []
    for k in range(N_CORES):
        r0 = RP * k
        l0 = LP * k

        x3h = np.empty((3, C_IN, RP + 2, W, B), np.float32)
        for kw in range(3):
            x3h[kw] = xp[:, :, r0 : r0 + RP + 2, kw : kw + W].transpose(1, 2, 3, 0)

        # W: [t, kh, (kw c), (lg o)]
        wk = wflat[:, :, :, l0 : l0 + LP, :].reshape(
            C_IN, 3, 3, NT, LG, C_OUT
        )  # (c, kh, kw, t, lg, o)
        wperm = wk.transpose(3, 1, 2, 0, 4, 5)  # (t, kh, kw, c, lg, o)
        wtile = wperm.reshape(NT, 3, 96, LG * C_OUT)

        # bias rows per group: (j, m, o)
        bk = bias_t[l0 : l0 + LP, :].reshape(NGRP, 4, 4, C_OUT)  # (gi, m, j, o)
        bi = bk.transpose(0, 2, 1, 3).reshape(NGRP, 512)  # (gi, (j, m, o))

        in_maps.append(
            {
                "x3h": x3h.reshape(96, XF).astype(NPDT),
                "w": np.ascontiguousarray(wtile).astype(NPDT),
                "bi": np.ascontiguousarray(bi).astype(NPDT),
                "oneh": oneh,
            }
        )
    return in_maps


def _get_nc():
    global _nc
    if _nc is None:
        _nc = _build()
    return _nc


def _gather(results):
    # out rows 32j+b (b<8) hold y[b, o, r, q] at col gi*128 + m*32 + o,
    # with r = gi//3, q = (gi%3)*16 + m*4 + j
    y = np.empty((B, C_OUT, H, W), np.float32)
    for k in range(N_CORES):
        arr = results[k]["out"].reshape(4, 32, NGRP, 4, C_OUT)  # (j, b*, gi, m, o)
        arr = arr[:, 0:B]  # (j, b, gi, m, o)
        arr = arr.transpose(1, 4, 2, 3, 0)  # (b, o, gi, m, j)
        arr = arr.reshape(B, C_OUT, RP, 3, 4, 4)  # (b, o, r, qg, m, j)
        y[:, :, RP * k : RP * (k + 1), :] = arr.reshape(B, C_OUT, RP, W)
    return y


def kernel(**inputs):
    nc = _get_nc()
    res = run_bass_kernel_spmd(nc, _shard(inputs), list(range(N_CORES)))
    return _gather(res.results)


# revision 17
# speedup vs baseline: 3.7872x; 1.0652x over previous
"""LocallyConnected2d (B=8, C_in=32, 48x48, C_out=32, 3x3, pad 1) on 8 trn2 cores.

Strategy: shard the spatial-location axis L = H*W across cores (6 image rows
each). Per location l the op is an (8x288)@(288x32) GEMM with location-unique
weights; weight streaming (85 MB total) dominates -> memory-bound.

Device mapping per core:
  - x halo slice lives in SBUF replicated 3x with kw column shifts, laid out
    [p=(kw*32+c), (row, col, b)], so the im2col patch for any location is a
    plain strided AP slice (no patch materialization).
  - Contraction (d=288) is split into 3 kh-rounds of K=96=(3 kw x 32 c),
    PSUM-accumulated. K=96 everywhere keeps one PE tiling mode (no drains);
    mixed-K designs either mode-switch per matmul or hit the "row tiles
    sharing a PSUM bank" hardware fault.
  - 4 consecutive locations are column-packed onto the PE with
    tile_position=(0, 32j): stationary = x-view [96, 8(b)] into column group
    j, moving = W slice [96, 32(o)], out = PSUM partitions 32j..32j+8. The
    four matmuls per (m, kh) execute concurrently on disjoint column groups.
  - Bias is added by one K=96 matmul per (group, j): a host-baked one-hot
    column picks the group's row out of a [96, 512] bias table (rows >=18
    zeroed on device), so the op shares the (128, 32) tiling mode.
  - W is host-permuted into per-(kh, LG-location) tiles that are fully
    contiguous in HBM with 9216-byte partition rows ([96, 9216B] DMAs
    measured ~193 GB/s vs ~94 GB/s naive); output is a [128, *] fp32 tile
    ((j,b) partitions x (group, m, o) free) stored with one fast DMA and
    unscrambled to NCHW on the host.
"""

import numpy as np

import concourse.bacc as bacc
import concourse.tile as tile
from concourse import mybir
from concourse.bass_utils import run_bass_kernel_spmd

B, C_IN, H, W = 8, 32, 48, 48
C_OUT = 32
N_CORES = 8
RP = H // N_CORES  # rows per core (6)
LP = RP * W  # locations per core (288)
NGRP = LP // 16  # 16-loc output groups per core (18)

DT16 = True  # fp16 operand path (halves weight traffic)
DT = mybir.dt.float16 if DT16 else mybir.dt.float32
NPDT = np.float16 if DT16 else np.float32
LG = 144 if DT16 else 96  # locs per W tile (9216B / 12KB partition rows)
NT = LP // LG  # W tiles per kh round
XF = (RP + 2) * W * B  # x3 free size (3072)
F32 = mybir.dt.float32

_nc = None


def _build():
    nc = bacc.Bacc(
        "TRN2", target_bir_lowering=False, debug=False, num_devices=N_CORES
    )
    x3h = nc.dram_tensor("x3h", [96, XF], DT, kind="ExternalInput")
    wts_d = nc.dram_tensor("w", [NT, 3, 96, LG * C_OUT], DT, kind="ExternalInput")
    bi = nc.dram_tensor("bi", [NGRP, 512], DT, kind="ExternalInput")
    oneh = nc.dram_tensor("oneh", [96, NGRP * 32], DT, kind="ExternalInput")
    out = nc.dram_tensor("out", [128, NGRP * 128], F32, kind="ExternalOutput")

    with tile.TileContext(nc) as tc:
        with (
            tc.tile_pool(name="xpool", bufs=1) as xpool,
            tc.tile_pool(name="wpool", bufs=6) as wpool,
            tc.tile_pool(name="opool", bufs=1) as opool,
            tc.tile_pool(name="pspool", bufs=8, space="PSUM") as pspool,
        ):
            x3 = xpool.tile([96, XF], DT, tag="x3")
            nc.gpsimd.dma_start(x3[:, :], x3h[:, :])
            oneh_sb = xpool.tile([96, NGRP * 32], DT, tag="oneh")
            nc.gpsimd.dma_start(oneh_sb[:, :], oneh[:, :])
            bi_sb = xpool.tile([96, 512], DT, tag="bi")
            nc.vector.memset(bi_sb[0:96, :], 0.0)
            nc.gpsimd.dma_start(bi_sb[0:NGRP, :], bi[:, :])

            out_sb = opool.tile([128, NGRP * 128], F32)

            for t in range(NT):
                wts = []
                for kh in range(3):
                    wt = wpool.tile([96, LG * C_OUT], DT, tag="wt")
                    nc.gpsimd.dma_start(wt[:, :], wts_d[t, kh, :, :])
                    wts.append(wt)
                for gl in range(LG // 16):
                    gi = t * (LG // 16) + gl
                    rl, qg = divmod(gi, 3)
                    ps = pspool.tile([128, 128], F32)
                    for j in range(4):
                        nc.tensor.matmul(
                            ps[32 * j : 32 * j + 32, 0:128],
                            oneh_sb[0:96, gi * 32 : gi * 32 + 32],
                            bi_sb[0:96, j * 128 : (j + 1) * 128],
                            start=True,
                            stop=False,
                            skip_group_check=True,
                            tile_position=(0, 32 * j),
                        )
                    for m in range(4):
                        for kh in range(3):
                            for j in range(4):
                                q = qg * 16 + m * 4 + j
                                l = rl * W + q
                                ll = l - t * LG
                                off = ((rl + kh) * W + q) * B
                                nc.tensor.matmul(
                                    ps[32 * j : 32 * j + B, m * 32 : (m + 1) * 32],
                                    x3[0:96, off : off + B],
                                    wts[kh][0:96, ll * 32 : (ll + 1) * 32],
                                    start=False,
                                    stop=(m == 3 and kh == 2),
                                    skip_group_check=True,
                                    tile_position=(0, 32 * j),
                                )
                    nc.vector.tensor_copy(
                        out_sb[0:128, gi * 128 : (gi + 1) * 128], ps[0:128, :]
                    )
            half = (NGRP // 2) * 128
            nc.gpsimd.dma_start(out[:, 0:half], out_sb[0:128, 0:half])
            nc.gpsimd.dma_start(out[:, half:], out_sb[0:128, half:])
    nc.compile()
    return nc


def _shard(inputs):
    x = np.asarray(inputs["x"], np.float32)
    weight = np.asarray(inputs["weight"], np.float32)[0]
    bias = np.asarray(inputs["bias"], np.float32)[0]
    xp = np.pad(x, ((0, 0), (0, 0), (1, 1), (1, 1)))  # (b, c, 50, 50)
    bias_t = bias.reshape(C_OUT, H * W).T  # (L, C_OUT)
    wflat = weight.reshape(C_IN, 3, 3, H * W, C_OUT)  # (c, kh, kw, l, o)

    # one-hot group selector [96, NGRP*32] (cols m>=8 zero)
    oneh = np.zeros((96, NGRP * 32), NPDT)
    for gi in range(NGRP):
        oneh[gi, gi * 32 : gi * 32 + 8] = 1.0

    in_maps = []
    for k in range(N_CORES):
        r0 = RP * k
        l0 = LP * k

        x3h = np.empty((3, C_IN, RP + 2, W, B), np.float32)
        for kw in range(3):
            x3h[kw] = xp[:, :, r0 : r0 + RP + 2, kw : kw + W].transpose(1, 2, 3, 0)

        # W: [t, kh, (kw c), (lg o)]
        wk = wflat[:, :, :, l0 : l0 + LP, :].reshape(
            C_IN, 3, 3, NT, LG, C_OUT
        )  # (c, kh, kw, t, lg, o)
        wperm = wk.transpose(3, 1, 2, 0, 4, 5)  # (t, kh, kw, c, lg, o)
        wtile = wperm.reshape(NT, 3, 96, LG * C_OUT)

        # bias rows per group: (j, m, o)
        bk = bias_t[l0 : l0 + LP, :].reshape(NGRP, 4, 4, C_OUT)  # (gi, m, j, o)
        bi = bk.transpose(0, 2, 1, 3).reshape(NGRP, 512)  # (gi, (j, m, o))

        in_maps.append(
            {
                "x3h": x3h.reshape(96, XF).astype(NPDT),
                "w": np.ascontiguousarray(wtile).astype(NPDT),
                "bi": np.ascontiguousarray(bi).astype(NPDT),
                "oneh": oneh,
            }
        )
    return in_maps


def _get_nc():
    global _nc
    if _nc is None:
        _nc = _build()
    return _nc


def _gather(results):
    # out rows 32j+b (b<8) hold y[b, o, r, q] at col gi*128 + m*32 + o,
    # with r = gi//3, q = (gi%3)*16 + m*4 + j
    y = np.empty((B, C_OUT, H, W), np.float32)
    for k in range(N_CORES):
        arr = results[k]["out"].reshape(4, 32, NGRP, 4, C_OUT)  # (j, b*, gi, m, o)
        arr = arr[:, 0:B]  # (j, b, gi, m, o)
        arr = arr.transpose(1, 4, 2, 3, 0)  # (b, o, gi, m, j)
        arr = arr.reshape(B, C_OUT, RP, 3, 4, 4)  # (b, o, r, qg, m, j)
        y[:, :, RP * k : RP * (k + 1), :] = arr.reshape(B, C_OUT, RP, W)
    return y


def kernel(**inputs):
    nc = _get_nc()
    res = run_bass_kernel_spmd(nc, _shard(inputs), list(range(N_CORES)))
    return _gather(res.results)


# revision 18
# speedup vs baseline: 3.8439x; 1.0150x over previous
"""LocallyConnected2d (B=8, C_in=32, 48x48, C_out=32, 3x3, pad 1) on 8 trn2 cores.

Strategy: shard the spatial-location axis L = H*W across cores (6 image rows
each). Per location l the op is an (8x288)@(288x32) GEMM with location-unique
weights; weight streaming (85 MB total) dominates -> memory-bound.

Device mapping per core:
  - x halo slice lives in SBUF replicated 3x with kw column shifts, laid out
    [p=(kw*32+c), (row, col, b)], so the im2col patch for any location is a
    plain strided AP slice (no patch materialization).
  - Contraction (d=288) is split into 3 kh-rounds of K=96=(3 kw x 32 c),
    PSUM-accumulated. K=96 everywhere keeps one PE tiling mode (no drains);
    mixed-K designs either mode-switch per matmul or hit the "row tiles
    sharing a PSUM bank" hardware fault.
  - 4 consecutive locations are column-packed onto the PE with
    tile_position=(0, 32j): stationary = x-view [96, 8(b)] into column group
    j, moving = W slice [96, 32(o)], out = PSUM partitions 32j..32j+8. The
    four matmuls per (m, kh) execute concurrently on disjoint column groups.
  - Bias is added by one K=96 matmul per (group, j): a host-baked one-hot
    column picks the group's row out of a [96, 512] bias table (rows >=18
    zeroed on device), so the op shares the (128, 32) tiling mode.
  - W is host-permuted into per-(kh, LG-location) tiles that are fully
    contiguous in HBM with 9216-byte partition rows ([96, 9216B] DMAs
    measured ~193 GB/s vs ~94 GB/s naive); output is a [128, *] fp32 tile
    ((j,b) partitions x (group, m, o) free) stored with one fast DMA and
    unscrambled to NCHW on the host.
"""

import numpy as np

import concourse.bacc as bacc
import concourse.tile as tile
from concourse import mybir
from concourse.bass_utils import run_bass_kernel_spmd

B, C_IN, H, W = 8, 32, 48, 48
C_OUT = 32
N_CORES = 8
RP = H // N_CORES  # rows per core (6)
LP = RP * W  # locations per core (288)
NGRP = LP // 16  # 16-loc output groups per core (18)

DT16 = True  # fp16 operand path (halves weight traffic)
DT = mybir.dt.float16 if DT16 else mybir.dt.float32
NPDT = np.float16 if DT16 else np.float32
LG = 48  # locs per W tile (all 3 kh rounds per tile)
NT = LP // LG  # W tiles (6)
SF = 0  # placeholder
XF = (RP + 2) * W * B  # x3 free size (3072)
F32 = mybir.dt.float32

_nc = None


def _build():
    nc = bacc.Bacc(
        "TRN2", target_bir_lowering=False, debug=False, num_devices=N_CORES
    )
    SF = XF + NGRP * 32 + 512  # combined static tile free size
    stat = nc.dram_tensor("stat", [96, SF], DT, kind="ExternalInput")
    wts_d = nc.dram_tensor("w", [NT, 96, 3 * LG * C_OUT], DT, kind="ExternalInput")
    out = nc.dram_tensor("out", [128, NGRP * 128], F32, kind="ExternalOutput")

    with tile.TileContext(nc) as tc:
        with (
            tc.tile_pool(name="xpool", bufs=1) as xpool,
            tc.tile_pool(name="wpool", bufs=4) as wpool,
            tc.tile_pool(name="opool", bufs=1) as opool,
            tc.tile_pool(name="pspool", bufs=8, space="PSUM") as pspool,
        ):
            stat_sb = xpool.tile([96, SF], DT, tag="stat")
            nc.gpsimd.dma_start(stat_sb[:, :], stat[:, :])
            x3 = stat_sb[:, 0:XF]
            oneh_sb = stat_sb[:, XF : XF + NGRP * 32]
            bi_sb = stat_sb[:, XF + NGRP * 32 : SF]

            out_sb = opool.tile([128, NGRP * 128], F32)

            for t in range(NT):
                wt = wpool.tile([96, 3 * LG * C_OUT], DT, tag="wt")
                nc.gpsimd.dma_start(wt[:, :], wts_d[t, :, :])
                for gl in range(LG // 16):
                    gi = t * (LG // 16) + gl
                    rl, qg = divmod(gi, 3)
                    ps = pspool.tile([128, 128], F32)
                    for j in range(4):
                        nc.tensor.matmul(
                            ps[32 * j : 32 * j + 32, 0:128],
                            oneh_sb[0:96, gi * 32 : gi * 32 + 32],
                            bi_sb[0:96, j * 128 : (j + 1) * 128],
                            start=True,
                            stop=False,
                            skip_group_check=True,
                            tile_position=(0, 32 * j),
                        )
                    for m in range(4):
                        for kh in range(3):
                            for j in range(4):
                                q = qg * 16 + m * 4 + j
                                l = rl * W + q
                                ll = l - t * LG
                                off = ((rl + kh) * W + q) * B
                                nc.tensor.matmul(
                                    ps[32 * j : 32 * j + B, m * 32 : (m + 1) * 32],
                                    x3[0:96, off : off + B],
                                    wt[0:96, (kh * LG + ll) * 32 : (kh * LG + ll + 1) * 32],
                                    start=False,
                                    stop=(m == 3 and kh == 2),
                                    skip_group_check=True,
                                    tile_position=(0, 32 * j),
                                )
                    nc.vector.tensor_copy(
                        out_sb[0:128, gi * 128 : (gi + 1) * 128], ps[0:128, :]
                    )
            for c0 in range(0, NGRP, 6):
                nc.gpsimd.dma_start(
                    out[:, c0 * 128 : (c0 + 6) * 128],
                    out_sb[0:128, c0 * 128 : (c0 + 6) * 128],
                )
    nc.compile()
    return nc


def _shard(inputs):
    x = np.asarray(inputs["x"], np.float32)
    weight = np.asarray(inputs["weight"], np.float32)[0]
    bias = np.asarray(inputs["bias"], np.float32)[0]
    xp = np.pad(x, ((0, 0), (0, 0), (1, 1), (1, 1)))  # (b, c, 50, 50)
    bias_t = bias.reshape(C_OUT, H * W).T  # (L, C_OUT)
    wflat = weight.reshape(C_IN, 3, 3, H * W, C_OUT)  # (c, kh, kw, l, o)

    # one-hot group selector [96, NGRP*32] (cols m>=8 zero)
    oneh = np.zeros((96, NGRP * 32), NPDT)
    for gi in range(NGRP):
        oneh[gi, gi * 32 : gi * 32 + 8] = 1.0

    in_maps = []
    for k in range(N_CORES):
        r0 = RP * k
        l0 = LP * k

        x3h = np.empty((3, C_IN, RP + 2, W, B), np.float32)
        for kw in range(3):
            x3h[kw] = xp[:, :, r0 : r0 + RP + 2, kw : kw + W].transpose(1, 2, 3, 0)

        # W: [t, (kw c), (kh lg o)]
        wk = wflat[:, :, :, l0 : l0 + LP, :].reshape(
            C_IN, 3, 3, NT, LG, C_OUT
        )  # (c, kh, kw, t, lg, o)
        wperm = wk.transpose(3, 2, 0, 1, 4, 5)  # (t, kw, c, kh, lg, o)
        wtile = wperm.reshape(NT, 96, 3 * LG * C_OUT)

        # bias rows per group: (j, m, o)
        bk = bias_t[l0 : l0 + LP, :].reshape(NGRP, 4, 4, C_OUT)  # (gi, m, j, o)
        bi = bk.transpose(0, 2, 1, 3).reshape(NGRP, 512)  # (gi, (j, m, o))

        stat = np.zeros((96, XF + NGRP * 32 + 512), NPDT)
        stat[:, 0:XF] = x3h.reshape(96, XF).astype(NPDT)
        stat[:, XF : XF + NGRP * 32] = oneh
        stat[0:NGRP, XF + NGRP * 32 :] = bi.astype(NPDT)
        in_maps.append(
            {
                "stat": stat,
                "w": np.ascontiguousarray(wtile).astype(NPDT),
            }
        )
    return in_maps


def _get_nc():
    global _nc
    if _nc is None:
        _nc = _build()
    return _nc


def _gather(results):
    # out rows 32j+b (b<8) hold y[b, o, r, q] at col gi*128 + m*32 + o,
    # with r = gi//3, q = (gi%3)*16 + m*4 + j
    y = np.empty((B, C_OUT, H, W), np.float32)
    for k in range(N_CORES):
        arr = results[k]["out"].reshape(4, 32, NGRP, 4, C_OUT)  # (j, b*, gi, m, o)
        arr = arr[:, 0:B]  # (j, b, gi, m, o)
        arr = arr.transpose(1, 4, 2, 3, 0)  # (b, o, gi, m, j)
        arr = arr.reshape(B, C_OUT, RP, 3, 4, 4)  # (b, o, r, qg, m, j)
        y[:, :, RP * k : RP * (k + 1), :] = arr.reshape(B, C_OUT, RP, W)
    return y


def kernel(**inputs):
    nc = _get_nc()
    res = run_bass_kernel_spmd(nc, _shard(inputs), list(range(N_CORES)))
    return _gather(res.results)


# revision 20
# speedup vs baseline: 3.8776x; 1.0088x over previous
"""LocallyConnected2d (B=8, C_in=32, 48x48, C_out=32, 3x3, pad 1) on 8 trn2 cores.

Strategy: shard the spatial-location axis L = H*W across cores (6 image rows
each). Per location l the op is an (8x288)@(288x32) GEMM with location-unique
weights; weight streaming (85 MB total) dominates -> memory-bound.

Device mapping per core:
  - x halo slice lives in SBUF replicated 3x with kw column shifts, laid out
    [p=(kw*32+c), (row, col, b)], so the im2col patch for any location is a
    plain strided AP slice (no patch materialization).
  - Contraction (d=288) is split into 3 kh-rounds of K=96=(3 kw x 32 c),
    PSUM-accumulated. K=96 everywhere keeps one PE tiling mode (no drains);
    mixed-K designs either mode-switch per matmul or hit the "row tiles
    sharing a PSUM bank" hardware fault.
  - 4 consecutive locations are column-packed onto the PE with
    tile_position=(0, 32j): stationary = x-view [96, 8(b)] into column group
    j, moving = W slice [96, 32(o)], out = PSUM partitions 32j..32j+8. The
    four matmuls per (m, kh) execute concurrently on disjoint column groups.
  - Bias is added by one K=96 matmul per (group, j): a host-baked one-hot
    column picks the group's row out of a [96, 512] bias table (rows >=18
    zeroed on device), so the op shares the (128, 32) tiling mode.
  - W is host-permuted into per-(kh, LG-location) tiles that are fully
    contiguous in HBM with 9216-byte partition rows ([96, 9216B] DMAs
    measured ~193 GB/s vs ~94 GB/s naive); output is a [128, *] fp32 tile
    ((j,b) partitions x (group, m, o) free) stored with one fast DMA and
    unscrambled to NCHW on the host.
"""

import numpy as np

import concourse.bacc as bacc
import concourse.tile as tile
from concourse import mybir
from concourse.bass_utils import run_bass_kernel_spmd

B, C_IN, H, W = 8, 32, 48, 48
C_OUT = 32
N_CORES = 8
RP = H // N_CORES  # rows per core (6)
LP = RP * W  # locations per core (288)
NGRP = LP // 16  # 16-loc output groups per core (18)

DT16 = True  # fp16 operand path (halves weight traffic)
DT = mybir.dt.float16 if DT16 else mybir.dt.float32
NPDT = np.float16 if DT16 else np.float32
LG = 48  # locs per W tile (all 3 kh rounds per tile)
NT = LP // LG  # W tiles (6)
SF = 0  # placeholder
XF = (RP + 2) * W * B  # x3 free size (3072)
F32 = mybir.dt.float32

_nc = None


def _build():
    nc = bacc.Bacc(
        "TRN2", target_bir_lowering=False, debug=False, num_devices=N_CORES
    )
    SF = XF + NGRP * 32 + 512  # combined static tile free size
    stat = nc.dram_tensor("stat", [96, SF], DT, kind="ExternalInput")
    wts_d = nc.dram_tensor("w", [NT, 96, 3 * LG * C_OUT], DT, kind="ExternalInput")
    out = nc.dram_tensor("out", [128, NGRP * 128], F32, kind="ExternalOutput")

    with tile.TileContext(nc) as tc:
        with (
            tc.tile_pool(name="xpool", bufs=1) as xpool,
            tc.tile_pool(name="wpool", bufs=4) as wpool,
            tc.tile_pool(name="opool", bufs=1) as opool,
            tc.tile_pool(name="pspool", bufs=8, space="PSUM") as pspool,
        ):
            stat_sb = xpool.tile([96, SF], DT, tag="stat")
            nc.gpsimd.dma_start(stat_sb[:, :], stat[:, :])
            x3 = stat_sb[:, 0:XF]
            oneh_sb = stat_sb[:, XF : XF + NGRP * 32]
            bi_sb = stat_sb[:, XF + NGRP * 32 : SF]

            out_sb = opool.tile([128, NGRP * 128], F32)

            for t in range(NT):
                wt = wpool.tile([96, 3 * LG * C_OUT], DT, tag="wt")
                nc.gpsimd.dma_start(wt[:, :], wts_d[t, :, :])
                for gl in range(LG // 16):
                    gi = t * (LG // 16) + gl
                    rl, qg = divmod(gi, 3)
                    ps = pspool.tile([128, 512], F32)
                    for j in range(4):
                        nc.tensor.matmul(
                            ps[32 * j : 32 * j + 32, 0:128],
                            oneh_sb[0:96, gi * 32 : gi * 32 + 32],
                            bi_sb[0:96, j * 128 : (j + 1) * 128],
                            start=True,
                            stop=False,
                            skip_group_check=True,
                            tile_position=(0, 32 * j),
                        )
                    for m in range(4):
                        for kh in range(3):
                            for j in range(4):
                                q = qg * 16 + m * 4 + j
                                l = rl * W + q
                                ll = l - t * LG
                                off = ((rl + kh) * W + q) * B
                                nc.tensor.matmul(
                                    ps[32 * j : 32 * j + B, m * 32 : (m + 1) * 32],
                                    x3[0:96, off : off + B],
                                    wt[0:96, (kh * LG + ll) * 32 : (kh * LG + ll + 1) * 32],
                                    start=False,
                                    stop=(m == 3 and kh == 2),
                                    skip_group_check=True,
                                    tile_position=(0, 32 * j),
                                )
                    nc.vector.tensor_copy(
                        out_sb[0:128, gi * 128 : (gi + 1) * 128], ps[0:128, 0:128]
                    )
            for c0 in range(0, NGRP, 6):
                nc.gpsimd.dma_start(
                    out[:, c0 * 128 : (c0 + 6) * 128],
                    out_sb[0:128, c0 * 128 : (c0 + 6) * 128],
                )
    nc.compile()
    return nc


def _shard(inputs):
    x = np.asarray(inputs["x"], np.float32)
    weight = np.asarray(inputs["weight"], np.float32)[0]
    bias = np.asarray(inputs["bias"], np.float32)[0]
    xp = np.pad(x, ((0, 0), (0, 0), (1, 1), (1, 1)))  # (b, c, 50, 50)
    bias_t = bias.reshape(C_OUT, H * W).T  # (L, C_OUT)
    wflat = weight.reshape(C_IN, 3, 3, H * W, C_OUT)  # (c, kh, kw, l, o)

    # one-hot group selector [96, NGRP*32] (cols m>=8 zero)
    oneh = np.zeros((96, NGRP * 32), NPDT)
    for gi in range(NGRP):
        oneh[gi, gi * 32 : gi * 32 + 8] = 1.0

    in_maps = []
    for k in range(N_CORES):
        r0 = RP * k
        l0 = LP * k

        x3h = np.empty((3, C_IN, RP + 2, W, B), np.float32)
        for kw in range(3):
            x3h[kw] = xp[:, :, r0 : r0 + RP + 2, kw : kw + W].transpose(1, 2, 3, 0)

        # W: [t, (kw c), (kh lg o)]
        wk = wflat[:, :, :, l0 : l0 + LP, :].reshape(
            C_IN, 3, 3, NT, LG, C_OUT
        )  # (c, kh, kw, t, lg, o)
        wperm = wk.transpose(3, 2, 0, 1, 4, 5)  # (t, kw, c, kh, lg, o)
        wtile = wperm.reshape(NT, 96, 3 * LG * C_OUT)

        # bias rows per group: (j, m, o)
        bk = bias_t[l0 : l0 + LP, :].reshape(NGRP, 4, 4, C_OUT)  # (gi, m, j, o)
        bi = bk.transpose(0, 2, 1, 3).reshape(NGRP, 512)  # (gi, (j, m, o))

        stat = np.zeros((96, XF + NGRP * 32 + 512), NPDT)
        stat[:, 0:XF] = x3h.reshape(96, XF).astype(NPDT)
        stat[:, XF : XF + NGRP * 32] = oneh
        stat[0:NGRP, XF + NGRP * 32 :] = bi.astype(NPDT)
        in_maps.append(
            {
                "stat": stat,
                "w": np.ascontiguousarray(wtile).astype(NPDT),
            }
        )
    return in_maps


def _get_nc():
    global _nc
    if _nc is None:
        _nc = _build()
    return _nc


def _gather(results):
    # out rows 32j+b (b<8) hold y[b, o, r, q] at col gi*128 + m*32 + o,
    # with r = gi//3, q = (gi%3)*16 + m*4 + j
    y = np.empty((B, C_OUT, H, W), np.float32)
    for k in range(N_CORES):
        arr = results[k]["out"].reshape(4, 32, NGRP, 4, C_OUT)  # (j, b*, gi, m, o)
        arr = arr[:, 0:B]  # (j, b, gi, m, o)
        arr = arr.transpose(1, 4, 2, 3, 0)  # (b, o, gi, m, j)
        arr = arr.reshape(B, C_OUT, RP, 3, 4, 4)  # (b, o, r, qg, m, j)
        y[:, :, RP * k : RP * (k + 1), :] = arr.reshape(B, C_OUT, RP, W)
    return y


def kernel(**inputs):
    nc = _get_nc()
    res = run_bass_kernel_spmd(nc, _shard(inputs), list(range(N_CORES)))
    return _gather(res.results)


# revision 21
# speedup vs baseline: 3.8831x; 1.0014x over previous
"""LocallyConnected2d (B=8, C_in=32, 48x48, C_out=32, 3x3, pad 1) on 8 trn2 cores.

Strategy: shard the spatial-location axis L = H*W across cores (6 image rows
each). Per location l the op is an (8x288)@(288x32) GEMM with location-unique
weights; weight streaming (85 MB total) dominates -> memory-bound.

Device mapping per core:
  - x halo slice lives in SBUF replicated 3x with kw column shifts, laid out
    [p=(kw*32+c), (row, col, b)], so the im2col patch for any location is a
    plain strided AP slice (no patch materialization).
  - Contraction (d=288) is split into 3 kh-rounds of K=96=(3 kw x 32 c),
    PSUM-accumulated. K=96 everywhere keeps one PE tiling mode (no drains);
    mixed-K designs either mode-switch per matmul or hit the "row tiles
    sharing a PSUM bank" hardware fault.
  - 4 consecutive locations are column-packed onto the PE with
    tile_position=(0, 32j): stationary = x-view [96, 8(b)] into column group
    j, moving = W slice [96, 32(o)], out = PSUM partitions 32j..32j+8. The
    four matmuls per (m, kh) execute concurrently on disjoint column groups.
  - Bias is added by one K=96 matmul per (group, j): a host-baked one-hot
    column picks the group's row out of a [96, 512] bias table (rows >=18
    zeroed on device), so the op shares the (128, 32) tiling mode.
  - W is host-permuted into per-(kh, LG-location) tiles that are fully
    contiguous in HBM with 9216-byte partition rows ([96, 9216B] DMAs
    measured ~193 GB/s vs ~94 GB/s naive); output is a [128, *] fp32 tile
    ((j,b) partitions x (group, m, o) free) stored with one fast DMA and
    unscrambled to NCHW on the host.
"""

import numpy as np

import concourse.bacc as bacc
import concourse.tile as tile
from concourse import mybir
from concourse.bass_utils import run_bass_kernel_spmd

B, C_IN, H, W = 8, 32, 48, 48
C_OUT = 32
N_CORES = 8
RP = H // N_CORES  # rows per core (6)
LP = RP * W  # locations per core (288)
NGRP = LP // 16  # 16-loc output groups per core (18)

DT16 = True  # fp16 operand path (halves weight traffic)
DT = mybir.dt.float16 if DT16 else mybir.dt.float32
NPDT = np.float16 if DT16 else np.float32
LG = 48  # locs per W tile (all 3 kh rounds per tile)
NT = LP // LG  # W tiles (6)
SF = 0  # placeholder
XF = (RP + 2) * W * B  # x3 free size (3072)
F32 = mybir.dt.float32

_nc = None


def _build():
    nc = bacc.Bacc(
        "TRN2", target_bir_lowering=False, debug=False, num_devices=N_CORES
    )
    SF = XF + NGRP * 32 + 512  # combined static tile free size
    stat = nc.dram_tensor("stat", [96, SF], DT, kind="ExternalInput")
    TILES = [(0, 16), (16, 32)] + [(48 * i, 48) for i in range(1, NT)]
    wds = [
        nc.dram_tensor(f"w{i}", [96, 3 * n * C_OUT], DT, kind="ExternalInput")
        for i, (_, n) in enumerate(TILES)
    ]
    out = nc.dram_tensor("out", [128, NGRP * 128], F32, kind="ExternalOutput")

    with tile.TileContext(nc) as tc:
        with (
            tc.tile_pool(name="xpool", bufs=1) as xpool,
            tc.tile_pool(name="wpool", bufs=4) as wpool,
            tc.tile_pool(name="opool", bufs=1) as opool,
            tc.tile_pool(name="pspool", bufs=8, space="PSUM") as pspool,
        ):
            stat_sb = xpool.tile([96, SF], DT, tag="stat")
            nc.gpsimd.dma_start(stat_sb[:, 0:XF], stat[:, 0:XF])
            nc.gpsimd.dma_start(stat_sb[:, XF:SF], stat[:, XF:SF])
            x3 = stat_sb[:, 0:XF]
            oneh_sb = stat_sb[:, XF : XF + NGRP * 32]
            bi_sb = stat_sb[:, XF + NGRP * 32 : SF]

            out_sb = opool.tile([128, NGRP * 128], F32)

            for t, (tl0, tn) in enumerate(TILES):
                wt = wpool.tile([96, 3 * 48 * C_OUT], DT, tag="wt")
                nc.gpsimd.dma_start(wt[0:96, 0 : 3 * tn * C_OUT], wds[t][:, :])
                for gl in range(tn // 16):
                    gi = tl0 // 16 + gl
                    rl, qg = divmod(gi, 3)
                    ps = pspool.tile([128, 512], F32)
                    for j in range(4):
                        nc.tensor.matmul(
                            ps[32 * j : 32 * j + 32, 0:128],
                            oneh_sb[0:96, gi * 32 : gi * 32 + 32],
                            bi_sb[0:96, j * 128 : (j + 1) * 128],
                            start=True,
                            stop=False,
                            skip_group_check=True,
                            tile_position=(0, 32 * j),
                        )
                    for m in range(4):
                        for kh in range(3):
                            for j in range(4):
                                q = qg * 16 + m * 4 + j
                                l = rl * W + q
                                ll = l - tl0
                                off = ((rl + kh) * W + q) * B
                                nc.tensor.matmul(
                                    ps[32 * j : 32 * j + B, m * 32 : (m + 1) * 32],
                                    x3[0:96, off : off + B],
                                    wt[0:96, (kh * tn + ll) * 32 : (kh * tn + ll + 1) * 32],
                                    start=False,
                                    stop=(m == 3 and kh == 2),
                                    skip_group_check=True,
                                    tile_position=(0, 32 * j),
                                )
                    nc.vector.tensor_copy(
                        out_sb[0:128, gi * 128 : (gi + 1) * 128], ps[0:128, 0:128]
                    )
            for c0 in range(0, NGRP, 6):
                nc.gpsimd.dma_start(
                    out[:, c0 * 128 : (c0 + 6) * 128],
                    out_sb[0:128, c0 * 128 : (c0 + 6) * 128],
                )
    nc.compile()
    return nc


def _shard(inputs):
    x = np.asarray(inputs["x"], np.float32)
    weight = np.asarray(inputs["weight"], np.float32)[0]
    bias = np.asarray(inputs["bias"], np.float32)[0]
    xp = np.pad(x, ((0, 0), (0, 0), (1, 1), (1, 1)))  # (b, c, 50, 50)
    bias_t = bias.reshape(C_OUT, H * W).T  # (L, C_OUT)
    wflat = weight.reshape(C_IN, 3, 3, H * W, C_OUT)  # (c, kh, kw, l, o)

    # one-hot group selector [96, NGRP*32] (cols m>=8 zero)
    oneh = np.zeros((96, NGRP * 32), NPDT)
    for gi in range(NGRP):
        oneh[gi, gi * 32 : gi * 32 + 8] = 1.0

    in_maps = []
    for k in range(N_CORES):
        r0 = RP * k
        l0 = LP * k

        x3h = np.empty((3, C_IN, RP + 2, W, B), np.float32)
        for kw in range(3):
            x3h[kw] = xp[:, :, r0 : r0 + RP + 2, kw : kw + W].transpose(1, 2, 3, 0)

        # W: per tile [(kw c), (kh, lg, o)]
        wk = wflat[:, :, :, l0 : l0 + LP, :]  # (c, kh, kw, LP, o)
        wall = wk.transpose(2, 0, 1, 3, 4).reshape(96, 3, LP, C_OUT)
        tiles = [(0, 16), (16, 32)] + [(48 * i, 48) for i in range(1, LP // 48)]
        wtiles = {
            f"w{i}": np.ascontiguousarray(
                wall[:, :, t0 : t0 + n, :].reshape(96, 3 * n * C_OUT)
            ).astype(NPDT)
            for i, (t0, n) in enumerate(tiles)
        }

        # bias rows per group: (j, m, o)
        bk = bias_t[l0 : l0 + LP, :].reshape(NGRP, 4, 4, C_OUT)  # (gi, m, j, o)
        bi = bk.transpose(0, 2, 1, 3).reshape(NGRP, 512)  # (gi, (j, m, o))

        stat = np.zeros((96, XF + NGRP * 32 + 512), NPDT)
        stat[:, 0:XF] = x3h.reshape(96, XF).astype(NPDT)
        stat[:, XF : XF + NGRP * 32] = oneh
        stat[0:NGRP, XF + NGRP * 32 :] = bi.astype(NPDT)
        m = {"stat": stat}
        m.update(wtiles)
        in_maps.append(m)
    return in_maps


def _get_nc():
    global _nc
    if _nc is None:
        _nc = _build()
    return _nc


def _gather(results):
    # out rows 32j+b (b<8) hold y[b, o, r, q] at col gi*128 + m*32 + o,
    # with r = gi//3, q = (gi%3)*16 + m*4 + j
    y = np.empty((B, C_OUT, H, W), np.float32)
    for k in range(N_CORES):
        arr = results[k]["out"].reshape(4, 32, NGRP, 4, C_OUT)  # (j, b*, gi, m, o)
        arr = arr[:, 0:B]  # (j, b, gi, m, o)
        arr = arr.transpose(1, 4, 2, 3, 0)  # (b, o, gi, m, j)
        arr = arr.reshape(B, C_OUT, RP, 3, 4, 4)  # (b, o, r, qg, m, j)
        y[:, :, RP * k : RP * (k + 1), :] = arr.reshape(B, C_OUT, RP, W)
    return y


def kernel(**inputs):
    nc = _get_nc()
    res = run_bass_kernel_spmd(nc, _shard(inputs), list(range(N_CORES)))
    return _gather(res.results)
